# revision 8
# baseline (speedup 1.0000x reference)
"""Tensor-parallel dense transformer (4-layer, D=1024, H=16, F=4096, S=2048,
V=32000 tied lm_head) on 8 Trainium2 NeuronCores via Bass/Tile.  v2.

Sharding (Megatron TP over 8 cores):
  - QKV: heads sharded -> 2 heads/core (EL=128); o_proj/down_proj input-dim
    sharded, f16 partial sums AllReduce'd; gate/up F-sharded (FL=512);
    lm_head vocab-sharded (VL=4000), host concat+transpose.

v2 vs v1:
  - residual stream `hid` kept d-major [128(d), ND, S] f32: no transpose DMAs;
    AllReduce readback adds directly; rmsnorm reductions over d (partitions)
    done on the PE with a ones-vector matmul, inv-rms broadcast via ones
    matmul.
  - whole layer body is emitted per 512-token s-chunk so each AllReduce
    (per GA-chunk group) overlaps compute of neighbouring chunks; causal
    attention makes chunk j depend only on KV chunks <= j.
  - engine rebalance: PSUM evacs split over Pool/ACT, Silu fused on ACT,
    residual adds split DVE/Pool.
  - lm_head emits v-major [VL, S] f16 logits (no strided DMA); host
    transposes and casts.

kernel(**inputs) takes the FULL unsharded inputs (as reference.setup_inputs)
and returns full logits [B, S, V] fp32.
"""
import os
import sys
sys.path.insert(0, "/opt/trn_rl_repo")

import numpy as np
import ml_dtypes
from contextlib import ExitStack

import concourse.bass as bass
import concourse.mybir as mybir
import concourse.tile as tile
from concourse import bacc
from concourse.bass import ts

BF = np.float16
F32 = mybir.dt.float32
F16 = mybir.dt.float16
AF = mybir.ActivationFunctionType
ALU = mybir.AluOpType

V, D, H, F, L, S, B = 32000, 1024, 16, 4096, 4, 2048, 1
NC_CORES = 8
ROPE_BASE = 10000.0
EPS = 1e-6
MASK_NEG = -30000.0
MOCK_CC = False          # replace collectives with local DMA (TimelineSim)
GA = int(os.environ.get("KGA", "2"))   # s-chunks per AllReduce group


def _dims():
    HD = 64
    HL = H // NC_CORES          # heads per core
    EL = HL * HD                # local qkv width
    FL = F // NC_CORES          # local ffn width
    VL = V // NC_CORES          # local vocab
    NT = S // 128               # 128-token tiles
    NSC = S // 512              # 512-token chunks
    ND = D // 128               # d chunks
    NFT = FL // 128             # f tiles
    return HD, HL, EL, FL, VL, NT, NSC, ND, NFT


def build_nc():
    HD, HL, EL, FL, VL, NT, NSC, ND, NFT = _dims()
    NG = NSC // GA              # AllReduce groups per pass
    SW = GA * 512               # tokens per group
    nc = bacc.Bacc("TRN2", target_bir_lowering=False, debug=False,
                   num_devices=NC_CORES)

    hid_ext = nc.dram_tensor("hidden0T", [D, S], F32, kind="ExternalInput")
    wq_ext = nc.dram_tensor("wqT", [L, D, EL], F16, kind="ExternalInput")
    wk_ext = nc.dram_tensor("wkT", [L, D, EL], F16, kind="ExternalInput")
    wv_ext = nc.dram_tensor("wvT", [L, D, EL], F16, kind="ExternalInput")
    wo_ext = nc.dram_tensor("woT", [L, EL, D], F16, kind="ExternalInput")
    wg_ext = nc.dram_tensor("wgT", [L, D, FL], F16, kind="ExternalInput")
    wu_ext = nc.dram_tensor("wuT", [L, D, FL], F16, kind="ExternalInput")
    wd_ext = nc.dram_tensor("wdT", [L, FL, D], F16, kind="ExternalInput")
    embT_ext = nc.dram_tensor("embT", [D, VL], F16, kind="ExternalInput")
    cos_ext = nc.dram_tensor("cosT", [EL, S], F16, kind="ExternalInput")
    sin_ext = nc.dram_tensor("sinT", [EL, S], F16, kind="ExternalInput")
    mask_ext = nc.dram_tensor("maskT", [4, 128, 512], F16, kind="ExternalInput")
    logits_ext = nc.dram_tensor("logitsT", [VL, S], F16, kind="ExternalOutput")

    cc_a_in = nc.dram_tensor("cc_a_in", [NG, D, SW], F16)
    cc_a_out = nc.dram_tensor("cc_a_out", [NG, D, SW], F16, addr_space="Shared")
    cc_f_in = nc.dram_tensor("cc_f_in", [NG, D, SW], F16)
    cc_f_out = nc.dram_tensor("cc_f_out", [NG, D, SW], F16, addr_space="Shared")
    RG = [list(range(NC_CORES))]

    with tile.TileContext(nc) as tc, ExitStack() as ctx:
        const_p = ctx.enter_context(tc.tile_pool(name="const", bufs=1))
        persist_p = ctx.enter_context(tc.tile_pool(name="persist", bufs=1))
        work_p = ctx.enter_context(tc.tile_pool(name="work", bufs=2))
        ps_main = ctx.enter_context(
            tc.tile_pool(name="psm", bufs=4, space="PSUM"))
        ps_sc = ctx.enter_context(
            tc.tile_pool(name="pssc", bufs=2, space="PSUM"))
        ps_av = ctx.enter_context(
            tc.tile_pool(name="psav", bufs=2, space="PSUM"))

        cos_sb = const_p.tile([EL, S], F16)
        nc.sync.dma_start(cos_sb[:], cos_ext[:, :])
        sin_sb = const_p.tile([EL, S], F16)
        nc.sync.dma_start(sin_sb[:], sin_ext[:, :])
        mask_sb = const_p.tile([128, 4, 512], F16)
        nc.sync.dma_start(mask_sb[:], mask_ext[:, :, :].rearrange("i p b -> p i b"))
        ones_row = const_p.tile([1, 128], F32)
        nc.gpsimd.memset(ones_row[:], 1.0)
        ones_col = const_p.tile([128, 1], F16)
        nc.gpsimd.memset(ones_col[:], 1.0)
        eps_sb = const_p.tile([1, 1], F32)
        nc.gpsimd.memset(eps_sb[:], EPS)

        hid = persist_p.tile([128, ND, S], F32)
        nc.sync.dma_start(hid[:], hid_ext[:, :].rearrange("(c p) s -> p c s", p=128))
        xT = persist_p.tile([128, ND, S], F16)
        qsb = persist_p.tile([EL, S], F16)
        ksb = persist_p.tile([EL, S], F16)
        o_in = persist_p.tile([EL, S], F16)
        v_store = persist_p.tile([128, NT, HL, 65], F16)
        nc.gpsimd.memset(v_store[:, :, :, 64:65], 1.0)

        def norm_chunk(j):
            """xT[:, :, j-chunk] = hid / rms(hid)  (d-major, PE reductions)."""
            jsl = ts(j, 512)
            # squares go into the xT chunk itself (scratch before the
            # normed values overwrite it)
            for dc in range(ND):
                if dc % 2 == 0:
                    nc.scalar.activation(xT[:, dc, jsl], hid[:, dc, jsl],
                                         AF.Square)
                else:
                    nc.vector.tensor_tensor(xT[:, dc, jsl], hid[:, dc, jsl],
                                            hid[:, dc, jsl], ALU.mult)
            ssq = ps_main.tile([128, 512], F32, tag="mm")
            for dc in range(ND):
                nc.tensor.matmul(ssq[0:1, :], ones_col[:, :], xT[:, dc, jsl],
                                 start=(dc == 0), stop=(dc == ND - 1))
            rms = work_p.tile([1, 512], F32, tag="rms")
            nc.scalar.activation(rms[:], ssq[0:1, :], AF.Sqrt, scale=1.0 / D,
                                 bias=eps_sb[:])
            inv = work_p.tile([1, 512], F32, tag="inv")
            nc.vector.reciprocal(inv[:], rms[:])
            bc = ps_main.tile([128, 512], F32, tag="mm")
            nc.tensor.matmul(bc[:], ones_row[:, :], inv[:], start=True, stop=True)
            invb = work_p.tile([128, 512], F16, tag="invb")
            nc.scalar.copy(invb[:], bc[:])
            for dc in range(ND):
                nc.vector.tensor_tensor(xT[:, dc, jsl], hid[:, dc, jsl],
                                        invb[:], ALU.mult)

        def resid_chunk(j, cc_out):
            """hid[:, :, j-chunk] += AllReduce output (d-major, no transpose)."""
            g, si = divmod(j, GA)
            jsl = ts(j, 512)
            for dc in range(ND):
                rb = work_p.tile([128, 512], F16, tag="rb", bufs=4)
                nc.sync.dma_start(rb[:], cc_out[g][ts(dc, 128), ts(si, 512)])
                eng = nc.vector if dc % 2 == 0 else nc.gpsimd
                eng.tensor_tensor(hid[:, dc, jsl], hid[:, dc, jsl], rb[:],
                                  ALU.add)

        def rope_chunk(src_ps, dst, j):
            # dst[:, jsl] = src*cos + perm(src)*sin_signed (32-blocks per head)
            sl = ts(j, 512)
            tq = work_p.tile([128, 512], F32, tag="ropet", bufs=1)
            nc.vector.tensor_tensor(tq[:], src_ps[:], cos_sb[:, sl], ALU.mult)
            u = work_p.tile([128, 512], F32, tag="ropeu", bufs=1)
            for h in range(HL):
                b = 64 * h
                nc.vector.tensor_tensor(u[b:b + 32, :], src_ps[b + 32:b + 64, :],
                                        sin_sb[b:b + 32, sl], ALU.mult)
                nc.vector.tensor_tensor(u[b + 32:b + 64, :], src_ps[b:b + 32, :],
                                        sin_sb[b + 32:b + 64, sl], ALU.mult)
            nc.vector.tensor_tensor(dst[:, sl], tq[:], u[:], ALU.add)

        def qkv_chunk(j, wq_sb, wk_sb, wv_sb):
            jsl = ts(j, 512)
            qps = ps_main.tile([128, 512], F32, tag="mm")
            for dc in range(ND):
                nc.tensor.matmul(qps[:], wq_sb[:, dc, :], xT[:, dc, jsl],
                                 start=(dc == 0), stop=(dc == ND - 1))
            rope_chunk(qps, qsb, j)
            kps = ps_main.tile([128, 512], F32, tag="mm")
            for dc in range(ND):
                nc.tensor.matmul(kps[:], wk_sb[:, dc, :], xT[:, dc, jsl],
                                 start=(dc == 0), stop=(dc == ND - 1))
            rope_chunk(kps, ksb, j)
            vps = ps_main.tile([128, 512], F32, tag="mm")
            for t4 in range(4):
                t = 4 * j + t4
                for dc in range(ND):
                    nc.tensor.matmul(vps[:, ts(t4, 128)], xT[:, dc, ts(t, 128)],
                                     wv_sb[:, dc, :],
                                     start=(dc == 0), stop=(dc == ND - 1))
            for t4 in range(4):
                for h in range(HL):
                    nc.scalar.copy(v_store[:, 4 * j + t4, h, 0:64],
                                   vps[:, 128 * t4 + 64 * h:128 * t4 + 64 * h + 64])

        def attn_chunk(j):
            kc_n = 4 * j + 4
            for h in range(HL):
                hb = 64 * h
                avp = ps_av.tile([65, 512], F32, tag="av")
                for kc in range(kc_n):
                    scp = ps_sc.tile([128, 512], F32, tag="sc")
                    nc.tensor.matmul(scp[:], ksb[hb:hb + 64, ts(kc, 128)],
                                     qsb[hb:hb + 64, ts(j, 512)],
                                     start=True, stop=True)
                    if kc >= 4 * j:
                        nc.vector.tensor_tensor(
                            scp[:], scp[:], mask_sb[:, kc - 4 * j, :], ALU.add)
                    psb = work_p.tile([128, 512], F16, tag="p", bufs=3)
                    nc.scalar.activation(psb[:], scp[:], AF.Exp, scale=0.125)
                    nc.tensor.matmul(avp[:], v_store[:, kc, h, :], psb[:],
                                     start=(kc == 0), stop=(kc == kc_n - 1))
                recip = work_p.tile([1, 512], F32, tag="recip")
                nc.vector.reciprocal(recip[:], avp[64:65, :])
                bcp = ps_main.tile([128, 512], F32, tag="mm")
                nc.tensor.matmul(bcp[0:64, :], ones_row[:, 0:64], recip[:],
                                 start=True, stop=True)
                bcsb = work_p.tile([64, 512], F16, tag="bcsb")
                nc.scalar.copy(bcsb[:], bcp[0:64, :])
                nc.vector.tensor_tensor(o_in[hb:hb + 64, ts(j, 512)],
                                        avp[0:64, :], bcsb[:], ALU.mult)

        def oproj_chunk(j, wo_sb):
            jsl = ts(j, 512)
            g, si = divmod(j, GA)
            for et in range(ND):
                ppt = ps_main.tile([128, 512], F32, tag="mm")
                nc.tensor.matmul(ppt[:], wo_sb[:, ts(et, 128)], o_in[:, jsl],
                                 start=True, stop=True)
                par = work_p.tile([128, 512], F16, tag="par", bufs=4)
                if et % 2 == 0:
                    nc.scalar.copy(par[:], ppt[:])
                else:
                    nc.vector.tensor_copy(par[:], ppt[:])
                nc.sync.dma_start(cc_a_in[g, ts(et, 128), ts(si, 512)], par[:])

        def ffn_chunk(j, wg_sb, wu_sb, wd_sb):
            jsl = ts(j, 512)
            g, si = divmod(j, GA)
            gsc = work_p.tile([128, NFT, 512], F16, tag="gsc")
            for ft in range(NFT):
                gps = ps_main.tile([128, 512], F32, tag="mm")
                for dc in range(ND):
                    nc.tensor.matmul(gps[:], wg_sb[:, dc, ts(ft, 128)],
                                     xT[:, dc, jsl],
                                     start=(dc == 0), stop=(dc == ND - 1))
                sg = work_p.tile([128, 512], F16, tag="sg")
                nc.scalar.activation(sg[:], gps[:], AF.Silu)
                ups = ps_main.tile([128, 512], F32, tag="mm")
                for dc in range(ND):
                    nc.tensor.matmul(ups[:], wu_sb[:, dc, ts(ft, 128)],
                                     xT[:, dc, jsl],
                                     start=(dc == 0), stop=(dc == ND - 1))
                nc.vector.tensor_tensor(gsc[:, ft, :], ups[:], sg[:], ALU.mult)
            for et in range(ND):
                dps = ps_main.tile([128, 512], F32, tag="mm")
                for fc in range(NFT):
                    nc.tensor.matmul(dps[:], wd_sb[:, fc, ts(et, 128)],
                                     gsc[:, fc, :],
                                     start=(fc == 0), stop=(fc == NFT - 1))
                par = work_p.tile([128, 512], F16, tag="par", bufs=4)
                nc.scalar.copy(par[:], dps[:])
                nc.sync.dma_start(cc_f_in[g, ts(et, 128), ts(si, 512)], par[:])

        def all_reduce(cc_in, cc_out, g):
            if MOCK_CC:
                nc.sync.dma_start(cc_out[g], cc_in[g])
            else:
                nc.gpsimd.collective_compute(
                    "AllReduce", ALU.add, replica_groups=RG,
                    ins=[cc_in[g].opt()], outs=[cc_out[g].opt()])

        # ---- initial pre-norm ----
        for j in range(NSC):
            norm_chunk(j)

        with ExitStack() as lctx:
            w_p = lctx.enter_context(tc.tile_pool(name="wts", bufs=2))
            wbig_p = lctx.enter_context(tc.tile_pool(name="wtsb", bufs=1))

            for l in range(L):
                wq_sb = w_p.tile([128, ND, EL], F16, tag="wq")
                nc.sync.dma_start(wq_sb[:], wq_ext[l].rearrange("(c p) e -> p c e", p=128))
                wk_sb = w_p.tile([128, ND, EL], F16, tag="wk")
                nc.sync.dma_start(wk_sb[:], wk_ext[l].rearrange("(c p) e -> p c e", p=128))
                wv_sb = w_p.tile([128, ND, EL], F16, tag="wv")
                nc.sync.dma_start(wv_sb[:], wv_ext[l].rearrange("(c p) e -> p c e", p=128))
                wo_sb = w_p.tile([EL, D], F16, tag="wo")
                nc.sync.dma_start(wo_sb[:], wo_ext[l])
                wg_sb = wbig_p.tile([128, ND, FL], F16, tag="wg")
                nc.sync.dma_start(wg_sb[:], wg_ext[l].rearrange("(c p) f -> p c f", p=128))
                wu_sb = wbig_p.tile([128, ND, FL], F16, tag="wu")
                nc.sync.dma_start(wu_sb[:], wu_ext[l].rearrange("(c p) f -> p c f", p=128))
                wd_sb = wbig_p.tile([128, NFT, D], F16, tag="wd")
                nc.sync.dma_start(wd_sb[:], wd_ext[l].rearrange("(c p) e -> p c e", p=128))

                for j in range(NSC):
                    qkv_chunk(j, wq_sb, wk_sb, wv_sb)
                for j in range(NSC):
                    attn_chunk(j)
                    oproj_chunk(j, wo_sb)
                    if (j + 1) % GA == 0:
                        all_reduce(cc_a_in, cc_a_out, j // GA)
                for j in range(NSC):
                    resid_chunk(j, cc_a_out)
                    norm_chunk(j)
                    ffn_chunk(j, wg_sb, wu_sb, wd_sb)
                    if (j + 1) % GA == 0:
                        all_reduce(cc_f_in, cc_f_out, j // GA)
                for j in range(NSC):
                    resid_chunk(j, cc_f_out)
                    norm_chunk(j)   # pre-norm for next layer / final norm

        # ---- lm_head (vocab-sharded, v-major output) ----
        with ExitStack() as ectx:
            emb_p = ectx.enter_context(tc.tile_pool(name="embp", bufs=1))
            VH = VL // 2
            for half in range(2):
                h0 = half * VH
                emb_sb = emb_p.tile([128, ND, VH], F16, tag="emb")
                nc.sync.dma_start(
                    emb_sb[:],
                    embT_ext[:, h0:h0 + VH].rearrange("(c p) v -> p c v", p=128))
                nvt = (VH + 127) // 128
                for sc in range(NSC):
                    ssl = ts(sc, 512)
                    for vt in range(nvt):
                        v0 = vt * 128
                        vn = min(128, VH - v0)
                        lp = ps_main.tile([128, 512], F32, tag="mm")
                        for dc in range(ND):
                            nc.tensor.matmul(lp[0:vn, :],
                                             emb_sb[:, dc, v0:v0 + vn],
                                             xT[:, dc, ssl],
                                             start=(dc == 0), stop=(dc == ND - 1))
                        lsb = work_p.tile([128, 512], F16, tag="lsb", bufs=2)
                        nc.scalar.copy(lsb[0:vn, :], lp[0:vn, :])
                        nc.sync.dma_start(
                            logits_ext[h0 + v0:h0 + v0 + vn, ssl], lsb[0:vn, :])

    nc.compile()
    return nc


def host_prep(inputs):
    """Full inputs -> per-core in_maps (list of dicts of np arrays)."""
    HD, HL, EL, FL, VL, NT, NSC, ND, NFT = _dims()
    emb = np.ascontiguousarray(np.asarray(inputs["emb"], np.float32))
    ids = np.asarray(inputs["input_ids"]).reshape(-1)
    hidden0T = np.ascontiguousarray(emb[ids].T).astype(np.float32)

    anw = np.asarray(inputs["attn_norm_w"], np.float32)
    fnw = np.asarray(inputs["ffn_norm_w"], np.float32)
    finw = np.asarray(inputs["final_norm_w"], np.float32)
    Wq = np.asarray(inputs["Wq"], np.float32)
    Wk = np.asarray(inputs["Wk"], np.float32)
    Wv = np.asarray(inputs["Wv"], np.float32)
    Wo = np.asarray(inputs["Wo"], np.float32)
    Wg = np.asarray(inputs["Wg"], np.float32)
    Wu = np.asarray(inputs["Wu"], np.float32)
    Wd = np.asarray(inputs["Wd"], np.float32)

    # rope tables [EL, S]
    inv_freq = 1.0 / (ROPE_BASE ** (np.arange(0, HD, 2, dtype=np.float32) / HD))
    ang = np.arange(S, dtype=np.float32)[:, None] * inv_freq[None, :]   # [S, HD/2]
    ang = np.concatenate([ang, ang], axis=1)                            # [S, HD]
    cosT = np.cos(ang).T.astype(np.float32)                             # [HD, S]
    sinT = np.sin(ang).T.astype(np.float32)
    sinT[:HD // 2] *= -1.0
    cos_full = np.tile(cosT, (HL, 1)).astype(BF)
    sin_full = np.tile(sinT, (HL, 1)).astype(BF)

    # causal masks [4, 128, 512]
    a = np.arange(128)[:, None]
    b = np.arange(512)[None, :]
    maskT = np.stack([(a + 128 * i > b) for i in range(4)]).astype(np.float32)
    maskT = (maskT * MASK_NEG).astype(BF)

    in_maps = []
    for c in range(NC_CORES):
        er = slice(c * EL, (c + 1) * EL)
        fr = slice(c * FL, (c + 1) * FL)
        vr = slice(c * VL, (c + 1) * VL)
        wqT = np.stack([(Wq[l][er, :] * anw[l][None, :]).T for l in range(L)])
        wkT = np.stack([(Wk[l][er, :] * anw[l][None, :]).T for l in range(L)])
        wvT = np.stack([(Wv[l][er, :] * anw[l][None, :]).T for l in range(L)])
        woT = np.stack([np.ascontiguousarray(Wo[l][:, er].T) for l in range(L)])
        wgT = np.stack([Wg[l][:, fr] * fnw[l][:, None] for l in range(L)])
        wuT = np.stack([Wu[l][:, fr] * fnw[l][:, None] for l in range(L)])
        wdT = np.stack([Wd[l][fr, :] for l in range(L)])
        embT = np.ascontiguousarray((emb[vr, :] * finw[None, :]).T)
        in_maps.append({
            "hidden0T": hidden0T,
            "wqT": wqT.astype(BF), "wkT": wkT.astype(BF), "wvT": wvT.astype(BF),
            "woT": woT.astype(BF), "wgT": wgT.astype(BF), "wuT": wuT.astype(BF),
            "wdT": wdT.astype(BF), "embT": embT.astype(BF),
            "cosT": cos_full, "sinT": sin_full, "maskT": maskT,
        })
    return in_maps


_RUNNER = None


def make_runner(nc):
    """Wrap a compiled Bacc module into a jitted 8-core callable."""
    import jax
    from jax.sharding import Mesh, PartitionSpec
    from jax.experimental.shard_map import shard_map
    from concourse.bass2jax import (_bass_exec_p, partition_id_tensor,
                                    install_neuronx_cc_hook)

    install_neuronx_cc_hook()

    partition_name = nc.partition_id_tensor.name if nc.partition_id_tensor else None
    in_names, out_names, out_avals = [], [], []
    for alloc in nc.m.functions[0].allocations:
        if not isinstance(alloc, mybir.MemoryLocationSet):
            continue
        name = alloc.memorylocations[0].name
        if alloc.kind == "ExternalInput":
            if name != partition_name:
                in_names.append(name)
        elif alloc.kind == "ExternalOutput":
            out_names.append(name)
            out_avals.append(jax.core.ShapedArray(
                tuple(alloc.tensor_shape), mybir.dt.np(alloc.dtype)))
    n_params = len(in_names)
    in_names_all = list(in_names) + list(out_names)
    if partition_name is not None:
        in_names_all.append(partition_name)

    def _body(*args):
        operands = list(args)
        if partition_name is not None:
            operands.append(partition_id_tensor())
        outs = _bass_exec_p.bind(
            *operands,
            out_avals=tuple(out_avals),
            in_names=tuple(in_names_all),
            out_names=tuple(out_names),
            lowering_input_output_aliases=(),
            sim_require_finite=True,
            sim_require_nnan=True,
            nc=nc,
        )
        return tuple(outs)

    devices = jax.devices()[:NC_CORES]
    mesh = Mesh(np.asarray(devices), ("core",))
    n_outs = len(out_names)
    in_specs = (PartitionSpec("core"),) * (n_params + n_outs)
    out_specs = (PartitionSpec("core"),) * len(out_names)
    sharded = jax.jit(shard_map(_body, mesh=mesh, in_specs=in_specs,
                                out_specs=out_specs, check_rep=False),
                      keep_unused=True)

    def zero_outs():
        return [np.zeros((NC_CORES * av.shape[0], *av.shape[1:]), av.dtype)
                for av in out_avals]

    def run(in_maps):
        concat_in = [np.concatenate([np.asarray(in_maps[c][nm])
                                     for c in range(NC_CORES)], axis=0)
                     for nm in in_names]
        out_arrs = sharded(*concat_in, *zero_outs())
        import jax as _jax
        _jax.block_until_ready(out_arrs)
        return [
            {nm: np.asarray(out_arrs[i]).reshape(NC_CORES, *out_avals[i].shape)[c]
             for i, nm in enumerate(out_names)}
            for c in range(NC_CORES)
        ]

    run.zero_outs = zero_outs
    run.sharded = sharded
    run.in_names = in_names
    run.out_names = out_names
    run.out_avals = out_avals
    run.mesh = mesh
    run.nc = nc
    return run


def _get_runner():
    """Build + compile the transformer once; cache the runner."""
    global _RUNNER
    if _RUNNER is None:
        _RUNNER = make_runner(build_nc())
    return _RUNNER


def kernel(**inputs) -> np.ndarray:
    HD, HL, EL, FL, VL, NT, NSC, ND, NFT = _dims()
    in_maps = host_prep(inputs)
    run = _get_runner()
    results = run(in_maps)
    logitsT = np.concatenate([results[c]["logitsT"] for c in range(NC_CORES)],
                             axis=0)          # [V, S] f16
    return logitsT.T.astype(np.float32).reshape(B, S, V)
